# revision 3
# baseline (speedup 1.0000x reference)
"""Window-routed sparse attention on 8 TRN2 NeuronCores.

Sharding: 64 windows x 8 cores = 8 windows/core (embarrassingly parallel).
Host precomputes the tiny routing path (region means, a_r [64,64]) and the
window-mixed q_m/k_m/v in fp32 numpy; each core runs the heavy windowed
attention relu(q_m k_m^T) v for its 8 windows on the Tensor engine in
float32r (full-rate fp32).
"""

import sys

sys.path.insert(0, "/opt/trn_rl_repo")

import numpy as np

C = 64          # channels
NW = 64         # windows (8x8 grid of 32x32 patches on 256x256)
T = 1024        # tokens per window (32*32)
NCORES = 8
WPC = NW // NCORES  # windows per core

_CACHE = {}


def _build_program():
    import concourse.mybir as mybir
    from concourse import bacc
    from concourse.tile import TileContext

    f32r = mybir.dt.float32r
    f32 = mybir.dt.float32

    nc = bacc.Bacc(None, target_bir_lowering=False)
    # c-major [c, i, t] for q_m/k_m; token-major [i, k, p, c] for v
    qm_d = nc.declare_dram_parameter("qm", [C, WPC, T], f32r, isOutput=False)
    km_d = nc.declare_dram_parameter("km", [C, WPC, T], f32r, isOutput=False)
    v_d = nc.declare_dram_parameter("v", [WPC, 8, 128, C], f32r, isOutput=False)
    o_d = nc.declare_dram_parameter("o", [C, WPC, T], f32r, isOutput=True)

    with TileContext(nc) as tc:
        with (
            tc.tile_pool(name="qk", bufs=2) as qk_pool,
            tc.tile_pool(name="vp", bufs=2) as v_pool,
            tc.tile_pool(name="at", bufs=2) as a_pool,
            tc.tile_pool(name="ob", bufs=2) as o_pool,
            tc.tile_pool(name="pa", bufs=2, space="PSUM") as pa_pool,
            tc.tile_pool(name="po", bufs=2, space="PSUM") as po_pool,
        ):
            for i in range(WPC):
                qm_t = qk_pool.tile([C, T], f32r, tag="qm")
                km_t = qk_pool.tile([C, T], f32r, tag="km")
                v_t = v_pool.tile([128, 8, C], f32r, tag="v")
                nc.sync.dma_start(out=qm_t, in_=qm_d[:, i, :])
                nc.sync.dma_start(out=km_t, in_=km_d[:, i, :])
                nc.sync.dma_start(out=v_t, in_=v_d[i].rearrange("k p c -> p k c"))

                # fused per s-chunk: qk matmul -> relu -> o accumulate.
                # single relu engine keeps per-instruction sync waits low.
                ps_o = po_pool.tile([C, T], f32, tag="pso")
                for k in range(8):
                    ps_a = pa_pool.tile([128, T], f32, tag="psa")
                    for h in range(2):
                        nc.tensor.matmul(
                            out=ps_a[:, h * 512:(h + 1) * 512],
                            lhsT=km_t[:, k * 128:(k + 1) * 128],
                            rhs=qm_t[:, h * 512:(h + 1) * 512],
                            start=True,
                            stop=True,
                        )
                    attn_t = a_pool.tile([128, T], f32r, tag="attn")
                    nc.scalar.activation(
                        out=attn_t,
                        in_=ps_a,
                        func=mybir.ActivationFunctionType.Relu,
                        scale=1.0,
                    )
                    for h in range(2):
                        nc.tensor.matmul(
                            out=ps_o[:, h * 512:(h + 1) * 512],
                            lhsT=v_t[:, k, :],
                            rhs=attn_t[:, h * 512:(h + 1) * 512],
                            start=(k == 0),
                            stop=(k == 7),
                        )
                o_t = o_pool.tile([C, T], f32r, tag="o")
                nc.vector.tensor_copy(out=o_t, in_=ps_o)
                nc.sync.dma_start(out=o_d[:, i, :], in_=o_t)

    nc.finalize()
    return nc


LAST_RESULT = None


def kernel(x, W, bias):
    import os
    from concourse.bass_utils import run_bass_kernel_spmd

    x = np.asarray(x, dtype=np.float32)
    W = np.asarray(W, dtype=np.float32)
    bias = np.asarray(bias, dtype=np.float32)

    # ---- host prep: windows, qkv, routing, mixing (tiny vs attention) ----
    # xw: [nw, T, c]
    xw = (
        x.reshape(C, 8, 32, 8, 32)
        .transpose(1, 3, 2, 4, 0)
        .reshape(NW, T, C)
    )
    qkv = xw @ W.T + bias  # [nw, T, 3c]
    q, k, v = qkv[..., :C], qkv[..., C:2 * C], qkv[..., 2 * C:]
    q_r = q.mean(axis=1)  # [nw, c]
    k_r = k.mean(axis=1)
    a_r = np.maximum(q_r @ k_r.T, 0.0)  # [nw, nw]
    k_m = np.tensordot(a_r, k, axes=(1, 0))  # [nw, T, c]
    q_m = np.tensordot(a_r, q, axes=(1, 0))

    if "nc" not in _CACHE:
        _CACHE["nc"] = _build_program()
    nc = _CACHE["nc"]

    in_maps = []
    for m in range(NCORES):
        s = slice(m * WPC, (m + 1) * WPC)
        in_maps.append({
            "qm": np.ascontiguousarray(q_m[s].transpose(2, 0, 1)),  # [c, wpc, T]
            "km": np.ascontiguousarray(k_m[s].transpose(2, 0, 1)),
            "v": np.ascontiguousarray(v[s].reshape(WPC, 8, 128, C)),
        })

    trace = bool(os.environ.get("KERNEL_TRACE"))
    res = run_bass_kernel_spmd(nc, in_maps, list(range(NCORES)), trace=trace)
    global LAST_RESULT
    LAST_RESULT = res
    outs = [res.results[m]["o"].reshape(C, WPC, T) for m in range(NCORES)]
    o_cm = np.concatenate(outs, axis=1)  # [c, nw, T]

    # fold back: [c, jh, jw, th, tw] -> [1, c, 256, 256]
    o_img = (
        o_cm.reshape(C, 8, 8, 32, 32)
        .transpose(0, 1, 3, 2, 4)
        .reshape(1, C, 256, 256)
    )
    return o_img.astype(np.float32)



# revision 4
# speedup vs baseline: 1.2092x; 1.2092x over previous
"""Window-routed sparse attention on 8 TRN2 NeuronCores.

Sharding: 64 windows x 8 cores = 8 windows/core (embarrassingly parallel).
Host precomputes the tiny routing path (region means, a_r [64,64]) and the
window-mixed q_m/k_m in fp32 numpy; each core runs the heavy windowed
attention relu(q_m k_m^T) v for its 8 windows on the Tensor engine.

v2: bf16 matmul operands (fp32r measured 4 cycles/row on HW; bf16 is 1),
relu+cast fused on alternating scalar/vector engines, fp32 PSUM accumulate.
"""

import sys

sys.path.insert(0, "/opt/trn_rl_repo")

import numpy as np
import ml_dtypes

BF16 = np.dtype(ml_dtypes.bfloat16)

C = 64          # channels
NW = 64         # windows (8x8 grid of 32x32 patches on 256x256)
T = 1024        # tokens per window (32*32)
NCORES = 8
WPC = NW // NCORES  # windows per core

_CACHE = {}

LAST_RESULT = None


def _build_program():
    import concourse.mybir as mybir
    from concourse import bacc
    from concourse.tile import TileContext

    bf16 = mybir.dt.bfloat16
    f32 = mybir.dt.float32

    nc = bacc.Bacc(None, target_bir_lowering=False)
    # c-major [c, i, t] for q_m/k_m; [i, p, k, c] for v (p=128 partition)
    qm_d = nc.declare_dram_parameter("qm", [C, WPC, T], bf16, isOutput=False)
    km_d = nc.declare_dram_parameter("km", [C, WPC, T], bf16, isOutput=False)
    v_d = nc.declare_dram_parameter("v", [WPC, 128, 8, C], bf16, isOutput=False)
    o_d = nc.declare_dram_parameter("o", [C, WPC, T], f32, isOutput=True)

    with TileContext(nc) as tc:
        with (
            tc.tile_pool(name="qk", bufs=2) as qk_pool,
            tc.tile_pool(name="vp", bufs=2) as v_pool,
            tc.tile_pool(name="at", bufs=3) as a_pool,
            tc.tile_pool(name="ob", bufs=2) as o_pool,
            tc.tile_pool(name="pa", bufs=2, space="PSUM") as pa_pool,
            tc.tile_pool(name="po", bufs=2, space="PSUM") as po_pool,
        ):
            for i in range(WPC):
                qm_t = qk_pool.tile([C, T], bf16, tag="qm")
                km_t = qk_pool.tile([C, T], bf16, tag="km")
                v_t = v_pool.tile([128, 8, C], bf16, tag="v")
                nc.sync.dma_start(out=qm_t, in_=qm_d[:, i, :])
                nc.sync.dma_start(out=km_t, in_=km_d[:, i, :])
                nc.sync.dma_start(out=v_t, in_=v_d[i])

                # fused per s-chunk: qk matmul -> relu(+bf16 cast) -> o accum.
                # relu alternates scalar/vector so neither ALU engine gates
                # the (now 4x faster) bf16 tensor stream.
                ps_o = po_pool.tile([C, T], f32, tag="pso")
                for k in range(8):
                    ps_a = pa_pool.tile([128, T], f32, tag="psa")
                    for h in range(2):
                        nc.tensor.matmul(
                            out=ps_a[:, h * 512:(h + 1) * 512],
                            lhsT=km_t[:, k * 128:(k + 1) * 128],
                            rhs=qm_t[:, h * 512:(h + 1) * 512],
                            start=True,
                            stop=True,
                        )
                    attn_t = a_pool.tile([128, T], bf16, tag="attn")
                    if k % 2 == 0:
                        nc.scalar.activation(
                            out=attn_t,
                            in_=ps_a,
                            func=mybir.ActivationFunctionType.Relu,
                            scale=1.0,
                        )
                    else:
                        nc.vector.tensor_scalar_max(attn_t, ps_a, 0.0)
                    for h in range(2):
                        nc.tensor.matmul(
                            out=ps_o[:, h * 512:(h + 1) * 512],
                            lhsT=v_t[:, k, :],
                            rhs=attn_t[:, h * 512:(h + 1) * 512],
                            start=(k == 0),
                            stop=(k == 7),
                        )
                o_t = o_pool.tile([C, T], f32, tag="o")
                if i % 2 == 0:
                    nc.vector.tensor_copy(out=o_t, in_=ps_o)
                else:
                    nc.scalar.copy(out=o_t, in_=ps_o)
                nc.sync.dma_start(out=o_d[:, i, :], in_=o_t)

    nc.finalize()
    return nc


def kernel(x, W, bias):
    import os
    from concourse.bass_utils import run_bass_kernel_spmd

    x = np.asarray(x, dtype=np.float32)
    W = np.asarray(W, dtype=np.float32)
    bias = np.asarray(bias, dtype=np.float32)

    # ---- host prep: windows, qkv, routing, mixing (tiny vs attention) ----
    # xw: [nw, T, c]
    xw = (
        x.reshape(C, 8, 32, 8, 32)
        .transpose(1, 3, 2, 4, 0)
        .reshape(NW, T, C)
    )
    qkv = xw @ W.T + bias  # [nw, T, 3c]
    q, k, v = qkv[..., :C], qkv[..., C:2 * C], qkv[..., 2 * C:]
    q_r = q.mean(axis=1)  # [nw, c]
    k_r = k.mean(axis=1)
    a_r = np.maximum(q_r @ k_r.T, 0.0)  # [nw, nw]
    k_m = np.tensordot(a_r, k, axes=(1, 0))  # [nw, T, c]
    q_m = np.tensordot(a_r, q, axes=(1, 0))

    if "nc" not in _CACHE:
        _CACHE["nc"] = _build_program()
    nc = _CACHE["nc"]

    # [nw, 128, 8, c]: s-chunk-of-128 minor-major layout for the o matmul
    v_dev = np.ascontiguousarray(
        v.reshape(NW, 8, 128, C).transpose(0, 2, 1, 3)
    ).astype(BF16)
    qm_dev = np.ascontiguousarray(q_m.transpose(2, 0, 1)).astype(BF16)  # [c,nw,T]
    km_dev = np.ascontiguousarray(k_m.transpose(2, 0, 1)).astype(BF16)

    in_maps = []
    for m in range(NCORES):
        s = slice(m * WPC, (m + 1) * WPC)
        in_maps.append({
            "qm": qm_dev[:, s, :],
            "km": km_dev[:, s, :],
            "v": v_dev[s],
        })

    trace = bool(os.environ.get("KERNEL_TRACE"))
    res = run_bass_kernel_spmd(nc, in_maps, list(range(NCORES)), trace=trace)
    global LAST_RESULT
    LAST_RESULT = res
    outs = [res.results[m]["o"].reshape(C, WPC, T) for m in range(NCORES)]
    o_cm = np.concatenate(outs, axis=1)  # [c, nw, T]

    # fold back: [c, jh, jw, th, tw] -> [1, c, 256, 256]
    o_img = (
        o_cm.reshape(C, 8, 8, 32, 32)
        .transpose(0, 1, 3, 2, 4)
        .reshape(1, C, 256, 256)
    )
    return o_img.astype(np.float32)


# revision 6
# speedup vs baseline: 1.8958x; 1.5678x over previous
"""Window-routed sparse attention on 8 TRN2 NeuronCores.

Sharding: 64 windows x 8 cores = 8 windows/core (embarrassingly parallel).
Host precomputes the tiny routing path (region means, a_r [64,64]) and the
window-mixed q_m/k_m in fp32 numpy; each core runs the heavy windowed
attention relu(q_m k_m^T) v for its 8 windows on the Tensor engine.

v2: bf16 matmul operands (fp32r measured 4 cycles/row on HW; bf16 is 1),
relu+cast fused on alternating scalar/vector engines, fp32 PSUM accumulate.
"""

import sys

sys.path.insert(0, "/opt/trn_rl_repo")

import numpy as np
import ml_dtypes

BF16 = np.dtype(ml_dtypes.bfloat16)

C = 64          # channels
NW = 64         # windows (8x8 grid of 32x32 patches on 256x256)
T = 1024        # tokens per window (32*32)
NCORES = 8
WPC = NW // NCORES  # windows per core

_CACHE = {}

LAST_RESULT = None


def _build_program():
    import concourse.mybir as mybir
    from concourse import bacc
    from concourse.tile import TileContext

    bf16 = mybir.dt.bfloat16
    f32 = mybir.dt.float32

    nc = bacc.Bacc(None, target_bir_lowering=False)
    # c-major [c, i, t] for q_m/k_m; [i, p, k, c] for v (p=128 partition)
    qm_d = nc.declare_dram_parameter("qm", [C, WPC, T], bf16, isOutput=False)
    km_d = nc.declare_dram_parameter("km", [C, WPC, T], bf16, isOutput=False)
    v_d = nc.declare_dram_parameter("v", [WPC, 128, 8, C], bf16, isOutput=False)
    o_d = nc.declare_dram_parameter("o", [C, WPC, T], f32, isOutput=True)

    with TileContext(nc) as tc:
        with (
            tc.tile_pool(name="qk", bufs=2) as qk_pool,
            tc.tile_pool(name="vp", bufs=2) as v_pool,
            tc.tile_pool(name="at", bufs=2) as a_pool,
            tc.tile_pool(name="ob", bufs=2) as o_pool,
            tc.tile_pool(name="pa", bufs=2, space="PSUM") as pa_pool,
            tc.tile_pool(name="po", bufs=2, space="PSUM") as po_pool,
        ):
            for i in range(WPC):
                qm_t = qk_pool.tile([C, T], bf16, tag="qm")
                km_t = qk_pool.tile([C, T], bf16, tag="km")
                v_t = v_pool.tile([128, 8, C], bf16, tag="v")
                nc.sync.dma_start(out=qm_t, in_=qm_d[:, i, :])
                nc.sync.dma_start(out=km_t, in_=km_d[:, i, :])
                nc.sync.dma_start(out=v_t, in_=v_d[i])

                # phase-split: all qk matmuls first (relus drain concurrently
                # to SBUF on alternating scalar/vector), then all o matmuls.
                # Keeps the PE stream continuous (p-state ramp) and off the
                # ALU critical path.
                ps_o = po_pool.tile([C, T], f32, tag="pso")
                at_w = a_pool.tile([128, 8, T], bf16, tag="attn")
                for k in range(8):
                    ps_a = pa_pool.tile([128, T], f32, tag="psa")
                    for h in range(2):
                        nc.tensor.matmul(
                            out=ps_a[:, h * 512:(h + 1) * 512],
                            lhsT=km_t[:, k * 128:(k + 1) * 128],
                            rhs=qm_t[:, h * 512:(h + 1) * 512],
                            start=True,
                            stop=True,
                        )
                    if k % 2 == 0:
                        nc.scalar.activation(
                            out=at_w[:, k, :],
                            in_=ps_a,
                            func=mybir.ActivationFunctionType.Relu,
                            scale=1.0,
                        )
                    else:
                        nc.vector.tensor_scalar_max(at_w[:, k, :], ps_a, 0.0)
                for k in range(8):
                    for h in range(2):
                        nc.tensor.matmul(
                            out=ps_o[:, h * 512:(h + 1) * 512],
                            lhsT=v_t[:, k, :],
                            rhs=at_w[:, k, h * 512:(h + 1) * 512],
                            start=(k == 0),
                            stop=(k == 7),
                        )
                o_t = o_pool.tile([C, T], f32, tag="o")
                if i % 2 == 0:
                    nc.vector.tensor_copy(out=o_t, in_=ps_o)
                else:
                    nc.scalar.copy(out=o_t, in_=ps_o)
                nc.sync.dma_start(out=o_d[:, i, :], in_=o_t)

    nc.finalize()
    return nc


def kernel(x, W, bias):
    import os
    from concourse.bass_utils import run_bass_kernel_spmd

    x = np.asarray(x, dtype=np.float32)
    W = np.asarray(W, dtype=np.float32)
    bias = np.asarray(bias, dtype=np.float32)

    # ---- host prep: windows, qkv, routing, mixing (tiny vs attention) ----
    # xw: [nw, T, c]
    xw = (
        x.reshape(C, 8, 32, 8, 32)
        .transpose(1, 3, 2, 4, 0)
        .reshape(NW, T, C)
    )
    qkv = xw @ W.T + bias  # [nw, T, 3c]
    q, k, v = qkv[..., :C], qkv[..., C:2 * C], qkv[..., 2 * C:]
    q_r = q.mean(axis=1)  # [nw, c]
    k_r = k.mean(axis=1)
    a_r = np.maximum(q_r @ k_r.T, 0.0)  # [nw, nw]
    k_m = np.tensordot(a_r, k, axes=(1, 0))  # [nw, T, c]
    q_m = np.tensordot(a_r, q, axes=(1, 0))

    if "nc" not in _CACHE:
        _CACHE["nc"] = _build_program()
    nc = _CACHE["nc"]

    # [nw, 128, 8, c]: s-chunk-of-128 minor-major layout for the o matmul
    v_dev = np.ascontiguousarray(
        v.reshape(NW, 8, 128, C).transpose(0, 2, 1, 3)
    ).astype(BF16)
    qm_dev = np.ascontiguousarray(q_m.transpose(2, 0, 1)).astype(BF16)  # [c,nw,T]
    km_dev = np.ascontiguousarray(k_m.transpose(2, 0, 1)).astype(BF16)

    in_maps = []
    for m in range(NCORES):
        s = slice(m * WPC, (m + 1) * WPC)
        in_maps.append({
            "qm": qm_dev[:, s, :],
            "km": km_dev[:, s, :],
            "v": v_dev[s],
        })

    trace = bool(os.environ.get("KERNEL_TRACE"))
    res = run_bass_kernel_spmd(nc, in_maps, list(range(NCORES)), trace=trace)
    global LAST_RESULT
    LAST_RESULT = res
    outs = [res.results[m]["o"].reshape(C, WPC, T) for m in range(NCORES)]
    o_cm = np.concatenate(outs, axis=1)  # [c, nw, T]

    # fold back: [c, jh, jw, th, tw] -> [1, c, 256, 256]
    o_img = (
        o_cm.reshape(C, 8, 8, 32, 32)
        .transpose(0, 1, 3, 2, 4)
        .reshape(1, C, 256, 256)
    )
    return o_img.astype(np.float32)


# revision 11
# speedup vs baseline: 2.0143x; 1.0625x over previous
"""Window-routed sparse attention on 8 TRN2 NeuronCores.

Sharding: 64 windows x 8 cores = 8 windows/core (embarrassingly parallel).
Host precomputes the tiny routing path (region means, a_r [64,64]) and the
window-mixed q_m/k_m in fp32 numpy; each core runs the heavy windowed
attention relu(q_m k_m^T) v for its 8 windows on the Tensor engine.

v2: bf16 matmul operands (fp32r measured 4 cycles/row on HW; bf16 is 1),
relu+cast fused on alternating scalar/vector engines, fp32 PSUM accumulate.
"""

import sys

sys.path.insert(0, "/opt/trn_rl_repo")

import numpy as np
import ml_dtypes

BF16 = np.dtype(ml_dtypes.bfloat16)

C = 64          # channels
NW = 64         # windows (8x8 grid of 32x32 patches on 256x256)
T = 1024        # tokens per window (32*32)
NCORES = 8
WPC = NW // NCORES  # windows per core

_CACHE = {}

LAST_RESULT = None


def _build_program():
    import concourse.mybir as mybir
    from concourse import bacc
    from concourse.tile import TileContext

    bf16 = mybir.dt.bfloat16
    f32 = mybir.dt.float32

    nc = bacc.Bacc(None, target_bir_lowering=False)
    # c-major [c, i, t] for q_m/k_m; [i, p, k, c] for v (p=128 partition)
    qm_d = nc.declare_dram_parameter("qm", [C, WPC, T], bf16, isOutput=False)
    km_d = nc.declare_dram_parameter("km", [C, WPC, T], bf16, isOutput=False)
    v_d = nc.declare_dram_parameter("v", [WPC, 128, 8, C], bf16, isOutput=False)
    o_d = nc.declare_dram_parameter("o", [C, WPC, T], f32, isOutput=True)

    with TileContext(nc) as tc:
        with (
            tc.tile_pool(name="qk", bufs=2) as qk_pool,
            tc.tile_pool(name="vp", bufs=2) as v_pool,
            tc.tile_pool(name="at", bufs=3) as a_pool,
            tc.tile_pool(name="ob", bufs=2) as o_pool,
            tc.tile_pool(name="pa", bufs=2, space="PSUM") as pa_pool,
            tc.tile_pool(name="po", bufs=2, space="PSUM") as po_pool,
        ):
            for i in range(WPC):
                qm_t = qk_pool.tile([C, T], bf16, tag="qm")
                km_t = qk_pool.tile([C, T], bf16, tag="km")
                v_t = v_pool.tile([128, 8, C], bf16, tag="v")
                nc.sync.dma_start(out=qm_t, in_=qm_d[:, i, :])
                nc.sync.dma_start(out=km_t, in_=km_d[:, i, :])
                nc.sync.dma_start(out=v_t, in_=v_d[i])

                # phase-split: all qk matmuls first (relus drain concurrently
                # to SBUF on alternating scalar/vector), then all o matmuls.
                # Keeps the PE stream continuous (p-state ramp) and off the
                # ALU critical path.
                ps_o = po_pool.tile([C, T], f32, tag="pso")
                at_w = a_pool.tile([128, 8, T], bf16, tag="attn")
                for k in range(8):
                    ps_a0 = pa_pool.tile([128, 512], f32, tag="psa0")
                    ps_a1 = pa_pool.tile([128, 512], f32, tag="psa1")
                    nc.tensor.matmul(
                        out=ps_a0,
                        lhsT=km_t[:, k * 128:(k + 1) * 128],
                        rhs=qm_t[:, 0:512],
                        start=True,
                        stop=True,
                    )
                    nc.tensor.matmul(
                        out=ps_a1,
                        lhsT=km_t[:, k * 128:(k + 1) * 128],
                        rhs=qm_t[:, 512:1024],
                        start=True,
                        stop=True,
                    )
                    nc.scalar.activation(
                        out=at_w[:, k, 0:512],
                        in_=ps_a0,
                        func=mybir.ActivationFunctionType.Relu,
                        scale=1.0,
                    )
                    nc.vector.tensor_scalar_max(
                        at_w[:, k, 512:1024], ps_a1, 0.0
                    )
                for k in range(8):
                    for h in range(2):
                        nc.tensor.matmul(
                            out=ps_o[:, h * 512:(h + 1) * 512],
                            lhsT=v_t[:, k, :],
                            rhs=at_w[:, k, h * 512:(h + 1) * 512],
                            start=(k == 0),
                            stop=(k == 7),
                        )
                o_t = o_pool.tile([C, T], f32, tag="o")
                if i % 2 == 0:
                    nc.vector.tensor_copy(out=o_t, in_=ps_o)
                else:
                    nc.scalar.copy(out=o_t, in_=ps_o)
                nc.sync.dma_start(out=o_d[:, i, :], in_=o_t)

    nc.finalize()
    return nc


def kernel(x, W, bias):
    import os
    from concourse.bass_utils import run_bass_kernel_spmd

    x = np.asarray(x, dtype=np.float32)
    W = np.asarray(W, dtype=np.float32)
    bias = np.asarray(bias, dtype=np.float32)

    # ---- host prep: windows, qkv, routing, mixing (tiny vs attention) ----
    # xw: [nw, T, c]
    xw = (
        x.reshape(C, 8, 32, 8, 32)
        .transpose(1, 3, 2, 4, 0)
        .reshape(NW, T, C)
    )
    qkv = xw @ W.T + bias  # [nw, T, 3c]
    q, k, v = qkv[..., :C], qkv[..., C:2 * C], qkv[..., 2 * C:]
    q_r = q.mean(axis=1)  # [nw, c]
    k_r = k.mean(axis=1)
    a_r = np.maximum(q_r @ k_r.T, 0.0)  # [nw, nw]
    k_m = np.tensordot(a_r, k, axes=(1, 0))  # [nw, T, c]
    q_m = np.tensordot(a_r, q, axes=(1, 0))

    if "nc" not in _CACHE:
        _CACHE["nc"] = _build_program()
    nc = _CACHE["nc"]

    # [nw, 128, 8, c]: s-chunk-of-128 minor-major layout for the o matmul
    v_dev = np.ascontiguousarray(
        v.reshape(NW, 8, 128, C).transpose(0, 2, 1, 3)
    ).astype(BF16)
    qm_dev = np.ascontiguousarray(q_m.transpose(2, 0, 1)).astype(BF16)  # [c,nw,T]
    km_dev = np.ascontiguousarray(k_m.transpose(2, 0, 1)).astype(BF16)

    in_maps = []
    for m in range(NCORES):
        s = slice(m * WPC, (m + 1) * WPC)
        in_maps.append({
            "qm": qm_dev[:, s, :],
            "km": km_dev[:, s, :],
            "v": v_dev[s],
        })

    trace = bool(os.environ.get("KERNEL_TRACE"))
    res = run_bass_kernel_spmd(nc, in_maps, list(range(NCORES)), trace=trace)
    global LAST_RESULT
    LAST_RESULT = res
    outs = [res.results[m]["o"].reshape(C, WPC, T) for m in range(NCORES)]
    o_cm = np.concatenate(outs, axis=1)  # [c, nw, T]

    # fold back: [c, jh, jw, th, tw] -> [1, c, 256, 256]
    o_img = (
        o_cm.reshape(C, 8, 8, 32, 32)
        .transpose(0, 1, 3, 2, 4)
        .reshape(1, C, 256, 256)
    )
    return o_img.astype(np.float32)


# revision 16
# speedup vs baseline: 2.0657x; 1.0255x over previous
"""Window-routed sparse attention on 8 TRN2 NeuronCores.

Sharding: 64 windows x 8 cores = 8 windows/core (embarrassingly parallel).
Host precomputes the tiny routing path (region means, a_r [64,64]) and the
window-mixed q_m/k_m in fp32 numpy; each core runs the heavy windowed
attention relu(q_m k_m^T) v for its 8 windows on the Tensor engine.

v2: bf16 matmul operands (fp32r measured 4 cycles/row on HW; bf16 is 1),
relu+cast fused on alternating scalar/vector engines, fp32 PSUM accumulate.
"""

import sys

sys.path.insert(0, "/opt/trn_rl_repo")

import numpy as np
import ml_dtypes

BF16 = np.dtype(ml_dtypes.bfloat16)

C = 64          # channels
NW = 64         # windows (8x8 grid of 32x32 patches on 256x256)
T = 1024        # tokens per window (32*32)
NCORES = 8
WPC = NW // NCORES  # windows per core

_CACHE = {}

LAST_RESULT = None


def _build_program():
    import concourse.mybir as mybir
    from concourse import bacc
    from concourse.tile import TileContext

    bf16 = mybir.dt.bfloat16
    f32 = mybir.dt.float32

    nc = bacc.Bacc(None, target_bir_lowering=False)
    # c-major [c, i, t] for q_m/k_m; [i, p, k, c] for v (p=128 partition)
    qm_d = nc.declare_dram_parameter("qm", [C, WPC, T], bf16, isOutput=False)
    km_d = nc.declare_dram_parameter("km", [C, WPC, T], bf16, isOutput=False)
    v_d = nc.declare_dram_parameter("v", [WPC, 128, 8, C], bf16, isOutput=False)
    o_d = nc.declare_dram_parameter("o", [C, WPC, T], f32, isOutput=True)

    with TileContext(nc) as tc:
        with (
            tc.tile_pool(name="qk", bufs=2) as qk_pool,
            tc.tile_pool(name="vp", bufs=2) as v_pool,
            tc.tile_pool(name="at", bufs=3) as a_pool,
            tc.tile_pool(name="ob", bufs=2) as o_pool,
            tc.tile_pool(name="pa", bufs=3, space="PSUM") as pa_pool,
            tc.tile_pool(name="po", bufs=1, space="PSUM") as po_pool,
        ):
            for i in range(WPC):
                qm_t = qk_pool.tile([C, T], bf16, tag="qm")
                km_t = qk_pool.tile([C, T], bf16, tag="km")
                v_t = v_pool.tile([128, 8, C], bf16, tag="v")
                nc.sync.dma_start(out=qm_t, in_=qm_d[:, i, :])
                nc.sync.dma_start(out=km_t, in_=km_d[:, i, :])
                nc.sync.dma_start(out=v_t, in_=v_d[i])

                # phase-split: all qk matmuls first (relus drain concurrently
                # to SBUF on alternating scalar/vector), then all o matmuls.
                # Keeps the PE stream continuous (p-state ramp) and off the
                # ALU critical path.
                ps_o = po_pool.tile([C, T], f32, tag="pso")
                at_w = a_pool.tile([128, 8, T], bf16, tag="attn")
                for k in range(8):
                    ps_a = pa_pool.tile([128, T], f32, tag="psa")
                    for h in range(2):
                        nc.tensor.matmul(
                            out=ps_a[:, h * 512:(h + 1) * 512],
                            lhsT=km_t[:, k * 128:(k + 1) * 128],
                            rhs=qm_t[:, h * 512:(h + 1) * 512],
                            start=True,
                            stop=True,
                        )
                    if k % 2 == 0:
                        nc.scalar.activation(
                            out=at_w[:, k, :],
                            in_=ps_a,
                            func=mybir.ActivationFunctionType.Relu,
                            scale=1.0,
                        )
                    else:
                        nc.vector.tensor_scalar_max(at_w[:, k, :], ps_a, 0.0)
                for k in range(8):
                    for h in range(2):
                        nc.tensor.matmul(
                            out=ps_o[:, h * 512:(h + 1) * 512],
                            lhsT=v_t[:, k, :],
                            rhs=at_w[:, k, h * 512:(h + 1) * 512],
                            start=(k == 0),
                            stop=(k == 7),
                        )
                o_t = o_pool.tile([C, T], f32, tag="o")
                if i % 2 == 0:
                    nc.vector.tensor_copy(out=o_t, in_=ps_o)
                else:
                    nc.scalar.copy(out=o_t, in_=ps_o)
                nc.sync.dma_start(out=o_d[:, i, :], in_=o_t)

    nc.finalize()
    return nc


def kernel(x, W, bias):
    import os
    from concourse.bass_utils import run_bass_kernel_spmd

    x = np.asarray(x, dtype=np.float32)
    W = np.asarray(W, dtype=np.float32)
    bias = np.asarray(bias, dtype=np.float32)

    # ---- host prep: windows, qkv, routing, mixing (tiny vs attention) ----
    # xw: [nw, T, c]
    xw = (
        x.reshape(C, 8, 32, 8, 32)
        .transpose(1, 3, 2, 4, 0)
        .reshape(NW, T, C)
    )
    qkv = xw @ W.T + bias  # [nw, T, 3c]
    q, k, v = qkv[..., :C], qkv[..., C:2 * C], qkv[..., 2 * C:]
    q_r = q.mean(axis=1)  # [nw, c]
    k_r = k.mean(axis=1)
    a_r = np.maximum(q_r @ k_r.T, 0.0)  # [nw, nw]
    k_m = np.tensordot(a_r, k, axes=(1, 0))  # [nw, T, c]
    q_m = np.tensordot(a_r, q, axes=(1, 0))

    if "nc" not in _CACHE:
        _CACHE["nc"] = _build_program()
    nc = _CACHE["nc"]

    # [nw, 128, 8, c]: s-chunk-of-128 minor-major layout for the o matmul
    v_dev = np.ascontiguousarray(
        v.reshape(NW, 8, 128, C).transpose(0, 2, 1, 3)
    ).astype(BF16)
    qm_dev = np.ascontiguousarray(q_m.transpose(2, 0, 1)).astype(BF16)  # [c,nw,T]
    km_dev = np.ascontiguousarray(k_m.transpose(2, 0, 1)).astype(BF16)

    in_maps = []
    for m in range(NCORES):
        s = slice(m * WPC, (m + 1) * WPC)
        in_maps.append({
            "qm": qm_dev[:, s, :],
            "km": km_dev[:, s, :],
            "v": v_dev[s],
        })

    trace = bool(os.environ.get("KERNEL_TRACE"))
    res = run_bass_kernel_spmd(nc, in_maps, list(range(NCORES)), trace=trace)
    global LAST_RESULT
    LAST_RESULT = res
    outs = [res.results[m]["o"].reshape(C, WPC, T) for m in range(NCORES)]
    o_cm = np.concatenate(outs, axis=1)  # [c, nw, T]

    # fold back: [c, jh, jw, th, tw] -> [1, c, 256, 256]
    o_img = (
        o_cm.reshape(C, 8, 8, 32, 32)
        .transpose(0, 1, 3, 2, 4)
        .reshape(1, C, 256, 256)
    )
    return o_img.astype(np.float32)
